# revision 34
# baseline (speedup 1.0000x reference)
"""Block-causal attention Trainium2 kernel (8 NeuronCores), v2.

Sharding: core c = b*4 + g handles batch b (of 2) and head-group g (4 of 16
heads). Each core computes the qkv projection, rmsnorm + 2-D RoPE,
block-causal attention and a partial output projection for its 256 channels;
the host sums the 4 per-group partials per batch (bf16 partials).

v2 changes vs baseline:
  rms sums via four zero-padded block-diagonal [128,8] stationaries -> one
    [8,512] psum bank (q heads rows 0:4, k heads rows 4:8, all mean-scaled so
    the Ln argument stays near 1 where the spline is accurate); rsqrt
    computed as exp(-0.5*ln(x+eps)) -- no sqrt table, and the 0.125 score
    scale is folded into the KPl multiply via scalar_tensor_tensor.
  normalize fused into the psum->sbuf copy (scalar_tensor_tensor with the
    broadcast reciprocal denominator, reciprocal_approx_fast on [33,512]);
    out-projection + output DMA run per-frame-pair inside the main loop
    (no serial tail); attention emitted via generators interleaved into
    phase 1 so its exp-paced stretches overlap projection matmuls.
  out, cos/sin tables, rope intermediates in bf16; shuffle DMAs on the
    sync/gpsimd queues (scalar queue kept free for the 160 exps).
Hardware pitfalls baked in: tile_position column-tiling (0,32)/(0,64)
  produces garbage on this HW (row-tiling is fine); multi-bank PSUM tiles
  misaddress the second bank; vector ops need 32-aligned partition bases and
  reciprocal_approx_fast silently fails on base-32 single-row slices.
PSUM: pps(2) + st(4x1) + pv(2) = 8 banks.
"""

import os
import numpy as np

import concourse.bass as bass
import concourse.mybir as mybir
import concourse.tile as tile
from concourse import bacc
from concourse.bass_utils import run_bass_kernel_spmd

F32 = mybir.dt.float32
BF16 = mybir.dt.bfloat16
AF = mybir.ActivationFunctionType
ALU = mybir.AluOpType
MUL = ALU.mult
ADD = ALU.add
SUB = ALU.subtract

B, T, NP, D, H = 2, 8, 256, 1024, 16
L = T * NP            # 2048
HD = 64               # head dim
HPG = 4               # heads per group (4 groups x 2 batches = 8 cores)
CPG = HPG * HD        # 256 channels per group
NDT = D // 128        # 8 d-tiles
NLC = L // 512        # 4 l-chunks (= frame pairs)
NLT = L // 128        # 16 l-tiles
EPS = 1e-6

_CACHE = {}


def _emit(nc, tc, ctx, xT, wqk, wv, wo, wvec, costab, sintab, out,
          skb36, skbq, skbd, dbg=None):
    sing = ctx.enter_context(tc.tile_pool(name="sing", bufs=1))
    xp = ctx.enter_context(tc.tile_pool(name="xp", bufs=8))
    tmp = ctx.enter_context(tc.tile_pool(name="tmp", bufs=3))
    sqp = ctx.enter_context(tc.tile_pool(name="sqp", bufs=4))
    ptp = ctx.enter_context(tc.tile_pool(name="ptp", bufs=5))
    osb = ctx.enter_context(tc.tile_pool(name="osb", bufs=2))
    bqp = ctx.enter_context(tc.tile_pool(name="bqp", bufs=4))
    rdp = ctx.enter_context(tc.tile_pool(name="rdp", bufs=2))
    # PSUM: pps 2 + st 4x1 + pv 2 = 8 banks
    pps = ctx.enter_context(tc.tile_pool(name="pps", bufs=2, space="PSUM"))
    stp = ctx.enter_context(tc.tile_pool(name="stp", bufs=4, space="PSUM"))
    pvp = ctx.enter_context(tc.tile_pool(name="pvp", bufs=2, space="PSUM"))

    # ---- persistent SBUF; first-needed first ----
    # wqk and the first l-chunk's x tiles interleaved per d-tile so the first
    # projection matmul can issue as soon as chunk 0 of each lands
    wqk_sb = sing.tile([128, NDT, 512], BF16)
    xt_pre = []
    for dt in range(NDT):
        x1 = xp.tile([128, 512], BF16, name=f"xtp{dt}", tag="xt")
        nc.sync.dma_start(out=x1[:], in_=xT[dt * 128:(dt + 1) * 128, 0:512])
        xt_pre.append(x1)
        nc.sync.dma_start(out=wqk_sb[:, dt, :],
                          in_=wqk[dt * 128:(dt + 1) * 128, :])
    wv_sb = sing.tile([128, NDT, CPG], BF16)
    nc.gpsimd.dma_start(out=wv_sb[:], in_=wv.rearrange("(t p) o -> p t o", p=128))
    wvec_sb = sing.tile([128, 32], BF16)
    nc.gpsimd.dma_start(out=wvec_sb[:], in_=wvec[:])
    cos_sb = sing.tile([128, L], BF16)
    nc.scalar.dma_start(out=cos_sb[:], in_=costab[:])
    sin_sb = sing.tile([128, L], BF16)
    nc.scalar.dma_start(out=sin_sb[:], in_=sintab[:])
    wo_sb = sing.tile([128, 2, D], BF16)
    nc.scalar.dma_start(out=wo_sb[:], in_=wo.rearrange("(t p) o -> p t o", p=128))

    qk_sb = [sing.tile([128, L], BF16, name=f"qk{i}") for i in range(4)]
    rope_sb = [sing.tile([128, L], BF16, name=f"rope{i}") for i in range(4)]
    QPl = [[sing.tile([128, 512], BF16, name=f"qp{i}_{c}") for c in range(NLC)]
           for i in range(2)]
    KPl = [[sing.tile([128, 512], BF16, name=f"kp{i}_{c}") for c in range(NLC)]
           for i in range(2)]
    v_sb = [sing.tile([128, NLT, 65], BF16, name=f"v{h}") for h in range(4)]
    att_sb = [sing.tile([128, L], BF16, name=f"att{i}") for i in range(2)]
    ones_f32 = sing.tile([128, NLT, 1], F32)
    nc.vector.memset(ones_f32[:], 1.0)
    for h in range(4):
        nc.vector.tensor_copy(v_sb[h][:, :, 64:65], ones_f32[:])


    epsb = sing.tile([8, 1], F32)
    nc.vector.memset(epsb[:], EPS)


    # ---------------- attention pass (generator, one head-pair) ------------
    def att2(hp, fp):
        nkt_sh, nkt_all = 4 * fp + 2, 4 * fp + 4
        last = nkt_all - 1
        fps = slice(512 * fp, 512 * fp + 512)
        pv = [pvp.tile([65, 512], F32, name=f"pv{hp}_{fp}_{i}", tag="pv")
              for i in range(2)]
        pend = []

        def flush_pv():
            kt_, pt_ = pend.pop(0)
            qof_ = 0 if kt_ < nkt_sh else 256
            nq_ = 512 if kt_ < nkt_sh else 256
            for i in range(2):
                nc.tensor.matmul(pv[i][:, qof_:512],
                                 v_sb[2 * hp + i][:, kt_, :],
                                 pt_[:, i, 0:nq_],
                                 start=(kt_ == 0), stop=(kt_ == last),
                                 skip_group_check=True)

        for kt in range(nkt_all):
            lck, kof = kt // 4, (kt % 4) * 128
            nq = 512 if kt < nkt_sh else 256
            qof = 0 if kt < nkt_sh else 256
            sts = [stp.tile([128, 512], F32, name=f"st{i}", tag="st")
                   for i in range(2)]
            for i in range(2):
                nc.tensor.matmul(sts[i][:, 0:nq],
                                 KPl[hp][lck][64 * i:64 * i + 64, kof:kof + 128],
                                 QPl[hp][fp][64 * i:64 * i + 64, qof:512],
                                 start=True, stop=True, skip_group_check=True)
            pt = ptp.tile([128, 2, 512], BF16, name="pt", tag="pt")
            for i in range(2):
                nc.scalar.activation(pt[:, i, 0:nq], sts[i][:, 0:nq], AF.Exp)
            pend.append((kt, pt))
            if len(pend) > 2:
                flush_pv()
            yield
        while pend:
            flush_pv()
            yield
        # extract denominators, normalize-fused copy to att_sb
        dstg = tmp.tile([33, 512], F32, tag="dc")
        nc.vector.memset(dstg[:], 1.0)
        for i in range(2):
            nc.vector.tensor_scalar(out=dstg[32 * i:32 * i + 1, :],
                                    in0=pv[i][64:65, :],
                                    scalar1=1e-30, scalar2=None, op0=ALU.max)
        dr = tmp.tile([33, 512], F32, tag="dc")
        nc.vector.reciprocal_approx_fast(out=dr[:], in_=dstg[:])
        for i in range(2):
            nc.sync.dma_start(out=skbd[2 * hp + i:2 * hp + i + 1, fps],
                              in_=dr[32 * i:32 * i + 1, :])
        for i in range(2):
            rd = rdp.tile([64, 512], F32, tag="rd")
            nc.sync.dma_start(out=rd[:],
                              in_=skbd[2 * hp + i:2 * hp + i + 1, fps]
                              .to_broadcast((64, 512)))
            nc.vector.scalar_tensor_tensor(out=att_sb[hp][64 * i:64 * i + 64, fps],
                                           in0=pv[i][0:64, :], scalar=1.0,
                                           in1=rd[:],
                                           op0=MUL, op1=MUL)
            if dbg is not None and hp == 0 and fp == 3:
                nc.sync.dma_start(out=dbg[0 + i, :], in_=dstg[32 * i:32 * i + 1, :])
                nc.sync.dma_start(out=dbg[2 + i, :], in_=dr[32 * i:32 * i + 1, :])
                nc.sync.dma_start(out=dbg[4 + i, :], in_=rd[0:1, :])
        yield

    def finish_fp(fp):
        for lt4 in range(4):
            lsl = slice((fp * 4 + lt4) * 128, (fp * 4 + lt4) * 128 + 128)
            for oc in range(2):
                ocs = slice(oc * 512, oc * 512 + 512)
                ps = pps.tile([128, 512], F32, name="ops", tag="ps")
                for ct in range(2):
                    nc.tensor.matmul(ps[:], att_sb[ct][:, lsl],
                                     wo_sb[:, ct, ocs], start=(ct == 0),
                                     stop=(ct == 1))
                ob = osb.tile([128, 512], BF16, tag="ob")
                nc.vector.tensor_copy(ob[:], ps[:])
                nc.sync.dma_start(out=out[lsl, ocs], in_=ob[:])
            yield

    def drive(gen, n):
        if gen is None:
            return False
        for _ in range(n):
            if next(gen, "END") == "END":
                return False
        return True

    # ---------------- main loop over l-chunks ------------------------------
    for lc in range(NLC):
        ls = slice(lc * 512, (lc + 1) * 512)
        # attention for the previous frame pair, interleaved into phase 1
        ag0 = att2(0, lc - 1) if lc >= 1 else None
        ag1 = att2(1, lc - 1) if lc >= 1 else None
        fin = finish_fp(lc - 1) if lc >= 1 else None
        nkt = 4 * lc  # kts per hp pass of fp=lc-1

        if lc == 0:
            xt = xt_pre
        else:
            xt = []
            for dt in range(NDT):
                x1 = xp.tile([128, 512], BF16, name=f"xt{dt}", tag="xt")
                nc.sync.dma_start(out=x1[:], in_=xT[dt * 128:(dt + 1) * 128, ls])
                xt.append(x1)

        sqs = []
        for pair in range(2):                    # 0: q (ot 0,1), 1: k (ot 2,3)
            drive(ag0, nkt // 2)
            for comp in range(2):
                ot = pair * 2 + comp
                ps = pps.tile([128, 512], F32, name="qkps", tag="ps")
                for dt in range(NDT):
                    nc.tensor.matmul(ps[:], wqk_sb[:, dt, ot * 128:(ot + 1) * 128],
                                     xt[dt][:], start=(dt == 0), stop=(dt == NDT - 1))
                nc.vector.tensor_copy(qk_sb[ot][:, ls], ps[:])
                sq = sqp.tile([128, 512], BF16, name="sq", tag="sq")
                nc.vector.tensor_tensor(sq[:], qk_sb[ot][:, ls],
                                        qk_sb[ot][:, ls], MUL)
                sqs.append(sq)
        drive(ag0, nkt // 2 + 1)

        # rms sums: one [8,512] bank; q heads rows 0:4, k heads rows 4:8
        rsum = pps.tile([8, 512], F32, name="rsum", tag="ps")
        for c in range(4):
            nc.tensor.matmul(rsum[:, :], wvec_sb[:, 8 * c:8 * c + 8],
                             sqs[c][:], start=(c == 0), stop=(c == 3),
                             skip_group_check=True)
        rln = tmp.tile([8, 512], F32, tag="rln")
        nc.scalar.activation(rln[:], rsum[:], AF.Ln, bias=epsb[:])
        rqs = tmp.tile([8, 512], BF16, tag="rqs")
        nc.scalar.activation(rqs[:], rln[:], AF.Exp, scale=-0.5)
        if dbg is not None and lc == 0:
            nc.sync.dma_start(out=dbg[6, :], in_=rqs[0:1, 0:512])
            nc.sync.dma_start(out=dbg[7, :], in_=rqs[4:5, 0:512])
        nc.gpsimd.dma_start(out=skbq[0:8, ls], in_=rqs[0:8, :])
        Rq_b = bqp.tile([128, 512], BF16, tag="bq")
        for h in range(4):
            nc.gpsimd.dma_start(out=Rq_b[32 * h:32 * h + 32, :],
                                in_=skbq[h:h + 1, ls].to_broadcast((32, 512)))
        Rk_b = []
        for hp in range(2):
            rkb = bqp.tile([128, 512], BF16, name=f"rkb{hp}", tag="bq")
            for i in range(2):
                nc.gpsimd.dma_start(
                    out=rkb[64 * i:64 * i + 64, :],
                    in_=skbq[4 + 2 * hp + i:5 + 2 * hp + i, ls]
                    .to_broadcast((64, 512)))
            Rk_b.append(rkb)

        # V projection: l on partitions; 2 chains share one bank
        drive(ag1, nkt // 2)
        for vt in range(2):
            vps = pps.tile([128, 2, CPG], F32, name="vps", tag="ps")
            for c2 in range(2):
                ls4 = vt * 2 + c2
                for dt in range(NDT):
                    nc.tensor.matmul(vps[:, c2, :],
                                     xt[dt][:, ls4 * 128:(ls4 + 1) * 128],
                                     wv_sb[:, dt, :],
                                     start=(c2 == 0 and dt == 0),
                                     stop=(c2 == 1 and dt == NDT - 1),
                                     skip_group_check=True)
            for c2 in range(2):
                lt = lc * 4 + vt * 2 + c2
                for h in range(4):
                    nc.vector.tensor_copy(v_sb[h][:, lt, 0:64],
                                          vps[:, c2, h * 64:(h + 1) * 64])
        drive(ag1, nkt // 2 + 1)
        # per-lc RoPE (+ r_q fold on the q side), all bf16
        for base in (0, 2):
            xr, xi = qk_sb[base][:, ls], qk_sb[base + 1][:, ls]
            for comp in range(2):
                t1 = tmp.tile([128, 512], BF16, tag="t1")
                t2 = tmp.tile([128, 512], BF16, tag="t2")
                ca, cb = (cos_sb, sin_sb) if comp == 0 else (sin_sb, cos_sb)
                nc.vector.tensor_tensor(t1[:], xr, ca[:, ls], MUL)
                nc.vector.tensor_tensor(t2[:], xi, cb[:, ls], MUL)
                op = SUB if comp == 0 else ADD
                dst = rope_sb[base + comp][:, ls]
                if base == 0:
                    t3 = tmp.tile([128, 512], BF16, tag="t3")
                    nc.vector.tensor_tensor(t3[:], t1[:], t2[:], op)
                    nc.vector.tensor_tensor(dst, t3[:], Rq_b[:], MUL)
                else:
                    nc.vector.tensor_tensor(dst, t1[:], t2[:], op)

        # shuffle into per-head contiguous tiles (sync queue)
        for hp2 in range(2):
            for i2 in range(2):
                h2 = hp2 * 2 + i2
                nc.gpsimd.dma_start(out=QPl[hp2][lc][64 * i2:64 * i2 + 32, :],
                                    in_=rope_sb[0][32 * h2:32 * h2 + 32, ls])
                nc.gpsimd.dma_start(out=QPl[hp2][lc][64 * i2 + 32:64 * i2 + 64, :],
                                    in_=rope_sb[1][32 * h2:32 * h2 + 32, ls])
                nc.gpsimd.dma_start(out=KPl[hp2][lc][64 * i2:64 * i2 + 32, :],
                                    in_=rope_sb[2][32 * h2:32 * h2 + 32, ls])
                nc.gpsimd.dma_start(out=KPl[hp2][lc][64 * i2 + 32:64 * i2 + 64, :],
                                    in_=rope_sb[3][32 * h2:32 * h2 + 32, ls])
        # fold 0.125*r_k into K (per head rows), in place
        for hp2 in range(2):
            nc.vector.scalar_tensor_tensor(out=KPl[hp2][lc][:],
                                           in0=KPl[hp2][lc][:], scalar=0.125,
                                           in1=Rk_b[hp2][:], op0=MUL, op1=MUL)

        # drain any remaining attention + the out-projection of fp=lc-1
        drive(ag0, 99)
        drive(ag1, 99)
        drive(fin, 99)

    # tail: fp = 3
    ag0, ag1 = att2(0, 3), att2(1, 3)
    drive(ag0, 99)
    drive(ag1, 99)
    drive(finish_fp(3), 99)


def _build_nc():
    import contextlib
    nc = bacc.Bacc("TRN2", target_bir_lowering=False, debug=False, num_devices=8)
    xT = nc.dram_tensor("xT", (D, L), BF16, kind="ExternalInput")
    wqk = nc.dram_tensor("wqk", (D, 512), BF16, kind="ExternalInput")
    wv = nc.dram_tensor("wv", (D, CPG), BF16, kind="ExternalInput")
    wo = nc.dram_tensor("wo", (CPG, D), BF16, kind="ExternalInput")
    wvec = nc.dram_tensor("wvec", (128, 32), BF16, kind="ExternalInput")
    costab = nc.dram_tensor("costab", (128, L), BF16, kind="ExternalInput")
    sintab = nc.dram_tensor("sintab", (128, L), BF16, kind="ExternalInput")
    out = nc.dram_tensor("out", (L, D), BF16, kind="ExternalOutput")
    skb36 = nc.dram_tensor("skb36", (36, L), F32)
    skbq = nc.dram_tensor("skbq", (8, L), BF16)
    skbd = nc.dram_tensor("skbd", (4, L), F32)
    dbg = (nc.dram_tensor("dbg", (8, 512), F32, kind="ExternalOutput")
           if os.environ.get("KERNEL_DBG") else None)

    with tile.TileContext(nc) as tc, contextlib.ExitStack() as ctx:
        _emit(nc, tc, ctx, xT.ap(), wqk.ap(), wv.ap(), wo.ap(), wvec.ap(),
              costab.ap(), sintab.ap(), out.ap(), skb36.ap(), skbq.ap(),
              skbd.ap(), dbg.ap() if dbg is not None else None)
    nc.compile()
    return nc


def _host_prep(x, Wqkv, Wout, q_scale, k_scale):
    x = np.asarray(x, np.float32)
    Wqkv = np.asarray(Wqkv, np.float32)
    Wout = np.asarray(Wout, np.float32)
    q_scale = np.asarray(q_scale, np.float32)
    k_scale = np.asarray(k_scale, np.float32)

    quarter = HD // 4  # 16
    inv = 1.0 / (10000.0 ** (np.arange(quarter, dtype=np.float64) / quarter))
    tt = np.repeat(np.arange(T), NP).astype(np.float64)
    pp = np.tile(np.arange(NP), T).astype(np.float64)
    ang = np.concatenate([tt[:, None] * inv[None, :], pp[:, None] * inv[None, :]],
                         axis=1)  # (L, 32)
    costab = np.tile(np.cos(ang).astype(np.float32).T, (4, 1))  # (128, L)
    sintab = np.tile(np.sin(ang).astype(np.float32).T, (4, 1))

    import ml_dtypes
    ev, od = np.arange(0, HD, 2), np.arange(1, HD, 2)
    # four [128,8] rms stationaries (qR,qI,kR,kI); q heads cols 0:4 of each
    # block, k heads cols 4:8; zero-padded so all mms share out rows 0:8
    wvec = np.zeros((128, 32), np.float32)
    for hh in range(HPG):
        r = slice(32 * hh, 32 * hh + 32)
        wvec[r, 0 + hh] = 1.0 / (HD * q_scale[ev] ** 2)
        wvec[r, 8 + hh] = 1.0 / (HD * q_scale[od] ** 2)
        wvec[r, 16 + 4 + hh] = 1.0 / (HD * k_scale[ev] ** 2)
        wvec[r, 24 + 4 + hh] = 1.0 / (HD * k_scale[od] ** 2)

    in_maps = []
    for c in range(8):
        b, g = c // 4, c % 4
        wqk = np.empty((D, 512), np.float32)
        for hh in range(HPG):
            gh = g * HPG + hh
            wq = Wqkv[gh * HD:(gh + 1) * HD, :] * q_scale[:, None]
            wk = Wqkv[D + gh * HD:D + (gh + 1) * HD, :] * k_scale[:, None]
            wqk[:, 0 + 32 * hh:32 + 32 * hh] = wq[ev].T
            wqk[:, 128 + 32 * hh:160 + 32 * hh] = wq[od].T
            wqk[:, 256 + 32 * hh:288 + 32 * hh] = wk[ev].T
            wqk[:, 384 + 32 * hh:416 + 32 * hh] = wk[od].T
        wv = np.ascontiguousarray(
            Wqkv[2 * D + g * CPG:2 * D + (g + 1) * CPG, :].T).astype(ml_dtypes.bfloat16)
        wo = np.ascontiguousarray(Wout[:, g * CPG:(g + 1) * CPG].T)
        in_maps.append({
            "xT": np.ascontiguousarray(x[b].T).astype(ml_dtypes.bfloat16),
            "wqk": wqk.astype(ml_dtypes.bfloat16), "wv": wv,
            "wo": wo.astype(ml_dtypes.bfloat16),
            "wvec": wvec.astype(ml_dtypes.bfloat16),
            "costab": costab.astype(ml_dtypes.bfloat16),
            "sintab": sintab.astype(ml_dtypes.bfloat16),
        })
    return in_maps


def kernel(x, Wqkv, Wout, q_scale, k_scale, T=None, N_p=None):
    assert int(T) == 8 and int(N_p) == 256
    if "nc" not in _CACHE:
        _CACHE["nc"] = _build_nc()
    nc = _CACHE["nc"]
    in_maps = _host_prep(x, Wqkv, Wout, q_scale, k_scale)
    trace = bool(int(os.environ.get("KERNEL_TRACE", "0")))
    res = run_bass_kernel_spmd(nc, in_maps, core_ids=list(range(8)), trace=trace)
    _CACHE["last_exec_time_ns"] = res.exec_time_ns
    outp = np.zeros((B, L, D), np.float32)
    for c in range(8):
        outp[c // 4] += np.asarray(res.results[c]["out"], np.float32)
    return outp


if __name__ == "__main__":
    rng = np.random.default_rng(0)
    x = rng.standard_normal((B, L, D), dtype=np.float32)
    Wqkv = rng.standard_normal((3 * D, D), dtype=np.float32) * 0.02
    Wout = rng.standard_normal((D, D), dtype=np.float32) * 0.02
    o = kernel(x, Wqkv, Wout, np.ones(HD, np.float32), np.ones(HD, np.float32),
               8, 256)
    print("out", o.shape, o.dtype, float(np.abs(o).mean()))


# revision 35
# speedup vs baseline: 1.0594x; 1.0594x over previous
"""Block-causal attention Trainium2 kernel (8 NeuronCores), v2.

Sharding: core c = b*4 + g handles batch b (of 2) and head-group g (4 of 16
heads). Each core computes the qkv projection, rmsnorm + 2-D RoPE,
block-causal attention and a partial output projection for its 256 channels;
the host sums the 4 per-group partials per batch (bf16 partials).

v2 changes vs baseline:
  rms sums via four zero-padded block-diagonal [128,8] stationaries -> one
    [8,512] psum bank (q heads rows 0:4, k heads rows 4:8, all mean-scaled so
    the Ln argument stays near 1 where the spline is accurate); rsqrt
    computed as exp(-0.5*ln(x+eps)) -- no sqrt table, and the 0.125 score
    scale is folded into the KPl multiply via scalar_tensor_tensor.
  normalize fused into the psum->sbuf copy (scalar_tensor_tensor with the
    broadcast reciprocal denominator, reciprocal_approx_fast on [33,512]);
    out-projection + output DMA run per-frame-pair inside the main loop
    (no serial tail); attention emitted via generators interleaved into
    phase 1 so its exp-paced stretches overlap projection matmuls.
  out, cos/sin tables, rope intermediates in bf16; shuffle DMAs on the
    sync/gpsimd queues (scalar queue kept free for the 160 exps).
Hardware pitfalls baked in: tile_position column-tiling (0,32)/(0,64)
  produces garbage on this HW (row-tiling is fine); multi-bank PSUM tiles
  misaddress the second bank; vector ops need 32-aligned partition bases and
  reciprocal_approx_fast silently fails on base-32 single-row slices.
PSUM: pps(2) + st(4x1) + pv(2) = 8 banks.
"""

import os
import numpy as np

import concourse.bass as bass
import concourse.mybir as mybir
import concourse.tile as tile
from concourse import bacc
from concourse.bass_utils import run_bass_kernel_spmd

F32 = mybir.dt.float32
BF16 = mybir.dt.bfloat16
AF = mybir.ActivationFunctionType
ALU = mybir.AluOpType
MUL = ALU.mult
ADD = ALU.add
SUB = ALU.subtract

B, T, NP, D, H = 2, 8, 256, 1024, 16
L = T * NP            # 2048
HD = 64               # head dim
HPG = 4               # heads per group (4 groups x 2 batches = 8 cores)
CPG = HPG * HD        # 256 channels per group
NDT = D // 128        # 8 d-tiles
NLC = L // 512        # 4 l-chunks (= frame pairs)
NLT = L // 128        # 16 l-tiles
EPS = 1e-6

_CACHE = {}


def _emit(nc, tc, ctx, xT, wqk, wv, wo, wvec, costab, sintab, out,
          skb36, skbq, skbd, dbg=None):
    sing = ctx.enter_context(tc.tile_pool(name="sing", bufs=1))
    xp = ctx.enter_context(tc.tile_pool(name="xp", bufs=8))
    tmp = ctx.enter_context(tc.tile_pool(name="tmp", bufs=3))
    sqp = ctx.enter_context(tc.tile_pool(name="sqp", bufs=4))
    ptp = ctx.enter_context(tc.tile_pool(name="ptp", bufs=5))
    osb = ctx.enter_context(tc.tile_pool(name="osb", bufs=2))
    bqp = ctx.enter_context(tc.tile_pool(name="bqp", bufs=4))
    rdp = ctx.enter_context(tc.tile_pool(name="rdp", bufs=2))
    # PSUM: pps 2 + st 4x1 + pv 2 = 8 banks
    pps = ctx.enter_context(tc.tile_pool(name="pps", bufs=2, space="PSUM"))
    stp = ctx.enter_context(tc.tile_pool(name="stp", bufs=4, space="PSUM"))
    pvp = ctx.enter_context(tc.tile_pool(name="pvp", bufs=2, space="PSUM"))

    # ---- persistent SBUF; first-needed first ----
    # wqk and the first l-chunk's x tiles interleaved per d-tile so the first
    # projection matmul can issue as soon as chunk 0 of each lands
    wqk_sb = sing.tile([128, NDT, 512], BF16)
    xt_pre = []
    for dt in range(NDT):
        x1 = xp.tile([128, 512], BF16, name=f"xtp{dt}", tag="xt")
        nc.sync.dma_start(out=x1[:], in_=xT[dt * 128:(dt + 1) * 128, 0:512])
        xt_pre.append(x1)
        nc.sync.dma_start(out=wqk_sb[:, dt, :],
                          in_=wqk[dt * 128:(dt + 1) * 128, :])
    wv_sb = sing.tile([128, NDT, CPG], BF16)
    nc.gpsimd.dma_start(out=wv_sb[:], in_=wv.rearrange("(t p) o -> p t o", p=128))
    wvec_sb = sing.tile([128, 32], BF16)
    nc.gpsimd.dma_start(out=wvec_sb[:], in_=wvec[:])
    cos_sb = sing.tile([128, L], BF16)
    nc.scalar.dma_start(out=cos_sb[:], in_=costab[:])
    sin_sb = sing.tile([128, L], BF16)
    nc.scalar.dma_start(out=sin_sb[:], in_=sintab[:])
    wo_sb = sing.tile([128, 2, D], BF16)
    nc.scalar.dma_start(out=wo_sb[:], in_=wo.rearrange("(t p) o -> p t o", p=128))

    qk_sb = [sing.tile([128, L], BF16, name=f"qk{i}") for i in range(4)]
    rope_sb = [sing.tile([128, L], BF16, name=f"rope{i}") for i in range(4)]
    QPl = [[sing.tile([128, 512], BF16, name=f"qp{i}_{c}") for c in range(NLC)]
           for i in range(2)]
    KPl = [[sing.tile([128, 512], BF16, name=f"kp{i}_{c}") for c in range(NLC)]
           for i in range(2)]
    v_sb = [sing.tile([128, NLT, 65], BF16, name=f"v{h}") for h in range(4)]
    att_sb = [sing.tile([128, L], BF16, name=f"att{i}") for i in range(2)]
    ones_f32 = sing.tile([128, NLT, 1], F32)
    nc.vector.memset(ones_f32[:], 1.0)
    for h in range(4):
        nc.vector.tensor_copy(v_sb[h][:, :, 64:65], ones_f32[:])


    epsb = sing.tile([8, 1], F32)
    nc.vector.memset(epsb[:], EPS)


    # ---------------- attention pass (generator, one head-pair) ------------
    def att2(hp, fp):
        nkt_sh, nkt_all = 4 * fp + 2, 4 * fp + 4
        last = nkt_all - 1
        fps = slice(512 * fp, 512 * fp + 512)
        pv = [pvp.tile([65, 512], F32, name=f"pv{hp}_{fp}_{i}", tag="pv")
              for i in range(2)]
        pend = []

        def flush_pv():
            kt_, pt_ = pend.pop(0)
            qof_ = 0 if kt_ < nkt_sh else 256
            nq_ = 512 if kt_ < nkt_sh else 256
            for i in range(2):
                nc.tensor.matmul(pv[i][:, qof_:512],
                                 v_sb[2 * hp + i][:, kt_, :],
                                 pt_[:, i, 0:nq_],
                                 start=(kt_ == 0), stop=(kt_ == last),
                                 skip_group_check=True)

        for kt in range(nkt_all):
            lck, kof = kt // 4, (kt % 4) * 128
            nq = 512 if kt < nkt_sh else 256
            qof = 0 if kt < nkt_sh else 256
            sts = [stp.tile([128, 512], F32, name=f"st{i}", tag="st")
                   for i in range(2)]
            for i in range(2):
                nc.tensor.matmul(sts[i][:, 0:nq],
                                 KPl[hp][lck][64 * i:64 * i + 64, kof:kof + 128],
                                 QPl[hp][fp][64 * i:64 * i + 64, qof:512],
                                 start=True, stop=True, skip_group_check=True)
            pt = ptp.tile([128, 2, 512], BF16, name="pt", tag="pt")
            for i in range(2):
                nc.scalar.activation(pt[:, i, 0:nq], sts[i][:, 0:nq], AF.Exp)
            pend.append((kt, pt))
            if len(pend) > 1:
                flush_pv()
            yield
        while pend:
            flush_pv()
            yield
        # extract denominators, normalize-fused copy to att_sb
        dstg = tmp.tile([33, 512], F32, tag="dc")
        nc.vector.memset(dstg[:], 1.0)
        for i in range(2):
            nc.vector.tensor_scalar(out=dstg[32 * i:32 * i + 1, :],
                                    in0=pv[i][64:65, :],
                                    scalar1=1e-30, scalar2=None, op0=ALU.max)
        dr = tmp.tile([33, 512], F32, tag="dc")
        nc.vector.reciprocal_approx_fast(out=dr[:], in_=dstg[:])
        for i in range(2):
            nc.sync.dma_start(out=skbd[2 * hp + i:2 * hp + i + 1, fps],
                              in_=dr[32 * i:32 * i + 1, :])
        for i in range(2):
            rd = rdp.tile([64, 512], F32, tag="rd")
            nc.sync.dma_start(out=rd[:],
                              in_=skbd[2 * hp + i:2 * hp + i + 1, fps]
                              .to_broadcast((64, 512)))
            nc.vector.scalar_tensor_tensor(out=att_sb[hp][64 * i:64 * i + 64, fps],
                                           in0=pv[i][0:64, :], scalar=1.0,
                                           in1=rd[:],
                                           op0=MUL, op1=MUL)
            if dbg is not None and hp == 0 and fp == 3:
                nc.sync.dma_start(out=dbg[0 + i, :], in_=dstg[32 * i:32 * i + 1, :])
                nc.sync.dma_start(out=dbg[2 + i, :], in_=dr[32 * i:32 * i + 1, :])
                nc.sync.dma_start(out=dbg[4 + i, :], in_=rd[0:1, :])
        yield

    def finish_fp(fp):
        for lt4 in range(4):
            lsl = slice((fp * 4 + lt4) * 128, (fp * 4 + lt4) * 128 + 128)
            for oc in range(2):
                ocs = slice(oc * 512, oc * 512 + 512)
                ps = pps.tile([128, 512], F32, name="ops", tag="ps")
                for ct in range(2):
                    nc.tensor.matmul(ps[:], att_sb[ct][:, lsl],
                                     wo_sb[:, ct, ocs], start=(ct == 0),
                                     stop=(ct == 1))
                ob = osb.tile([128, 512], BF16, tag="ob")
                nc.vector.tensor_copy(ob[:], ps[:])
                nc.sync.dma_start(out=out[lsl, ocs], in_=ob[:])
            yield

    def drive(gen, n):
        if gen is None:
            return False
        for _ in range(n):
            if next(gen, "END") == "END":
                return False
        return True

    # ---------------- main loop over l-chunks ------------------------------
    for lc in range(NLC):
        ls = slice(lc * 512, (lc + 1) * 512)
        # attention for the previous frame pair, interleaved into phase 1
        ag0 = att2(0, lc - 1) if lc >= 1 else None
        ag1 = att2(1, lc - 1) if lc >= 1 else None
        fin = finish_fp(lc - 1) if lc >= 1 else None
        nkt = 4 * lc  # kts per hp pass of fp=lc-1

        if lc == 0:
            xt = xt_pre
        else:
            xt = []
            for dt in range(NDT):
                x1 = xp.tile([128, 512], BF16, name=f"xt{dt}", tag="xt")
                nc.sync.dma_start(out=x1[:], in_=xT[dt * 128:(dt + 1) * 128, ls])
                xt.append(x1)

        sqs = []
        for pair in range(2):                    # 0: q (ot 0,1), 1: k (ot 2,3)
            drive(ag0, nkt // 2)
            for comp in range(2):
                ot = pair * 2 + comp
                ps = pps.tile([128, 512], F32, name="qkps", tag="ps")
                for dt in range(NDT):
                    nc.tensor.matmul(ps[:], wqk_sb[:, dt, ot * 128:(ot + 1) * 128],
                                     xt[dt][:], start=(dt == 0), stop=(dt == NDT - 1))
                nc.vector.tensor_copy(qk_sb[ot][:, ls], ps[:])
                sq = sqp.tile([128, 512], BF16, name="sq", tag="sq")
                nc.vector.tensor_tensor(sq[:], qk_sb[ot][:, ls],
                                        qk_sb[ot][:, ls], MUL)
                sqs.append(sq)
        drive(ag0, nkt // 2 + 1)

        # rms sums: one [8,512] bank; q heads rows 0:4, k heads rows 4:8
        rsum = pps.tile([8, 512], F32, name="rsum", tag="ps")
        for c in range(4):
            nc.tensor.matmul(rsum[:, :], wvec_sb[:, 8 * c:8 * c + 8],
                             sqs[c][:], start=(c == 0), stop=(c == 3),
                             skip_group_check=True)
        rln = tmp.tile([8, 512], F32, tag="rln")
        nc.scalar.activation(rln[:], rsum[:], AF.Ln, bias=epsb[:])
        rqs = tmp.tile([8, 512], BF16, tag="rqs")
        nc.scalar.activation(rqs[:], rln[:], AF.Exp, scale=-0.5)
        if dbg is not None and lc == 0:
            nc.sync.dma_start(out=dbg[6, :], in_=rqs[0:1, 0:512])
            nc.sync.dma_start(out=dbg[7, :], in_=rqs[4:5, 0:512])
        nc.gpsimd.dma_start(out=skbq[0:8, ls], in_=rqs[0:8, :])
        Rq_b = bqp.tile([128, 512], BF16, tag="bq")
        for h in range(4):
            nc.gpsimd.dma_start(out=Rq_b[32 * h:32 * h + 32, :],
                                in_=skbq[h:h + 1, ls].to_broadcast((32, 512)))
        Rk_b = []
        for hp in range(2):
            rkb = bqp.tile([128, 512], BF16, name=f"rkb{hp}", tag="bq")
            for i in range(2):
                nc.gpsimd.dma_start(
                    out=rkb[64 * i:64 * i + 64, :],
                    in_=skbq[4 + 2 * hp + i:5 + 2 * hp + i, ls]
                    .to_broadcast((64, 512)))
            Rk_b.append(rkb)

        # V projection: l on partitions; 2 chains share one bank
        drive(ag1, nkt // 2)
        for vt in range(2):
            vps = pps.tile([128, 2, CPG], F32, name="vps", tag="ps")
            for c2 in range(2):
                ls4 = vt * 2 + c2
                for dt in range(NDT):
                    nc.tensor.matmul(vps[:, c2, :],
                                     xt[dt][:, ls4 * 128:(ls4 + 1) * 128],
                                     wv_sb[:, dt, :],
                                     start=(c2 == 0 and dt == 0),
                                     stop=(c2 == 1 and dt == NDT - 1),
                                     skip_group_check=True)
            for c2 in range(2):
                lt = lc * 4 + vt * 2 + c2
                for h in range(4):
                    nc.vector.tensor_copy(v_sb[h][:, lt, 0:64],
                                          vps[:, c2, h * 64:(h + 1) * 64])
        drive(ag1, nkt // 2 + 1)
        # per-lc RoPE (+ r_q fold on the q side), all bf16
        for base in (0, 2):
            xr, xi = qk_sb[base][:, ls], qk_sb[base + 1][:, ls]
            for comp in range(2):
                t1 = tmp.tile([128, 512], BF16, tag="t1")
                t2 = tmp.tile([128, 512], BF16, tag="t2")
                ca, cb = (cos_sb, sin_sb) if comp == 0 else (sin_sb, cos_sb)
                nc.vector.tensor_tensor(t1[:], xr, ca[:, ls], MUL)
                nc.vector.tensor_tensor(t2[:], xi, cb[:, ls], MUL)
                op = SUB if comp == 0 else ADD
                dst = rope_sb[base + comp][:, ls]
                if base == 0:
                    t3 = tmp.tile([128, 512], BF16, tag="t3")
                    nc.vector.tensor_tensor(t3[:], t1[:], t2[:], op)
                    nc.vector.tensor_tensor(dst, t3[:], Rq_b[:], MUL)
                else:
                    nc.vector.tensor_tensor(dst, t1[:], t2[:], op)

        # shuffle into per-head contiguous tiles (sync queue)
        for hp2 in range(2):
            for i2 in range(2):
                h2 = hp2 * 2 + i2
                nc.gpsimd.dma_start(out=QPl[hp2][lc][64 * i2:64 * i2 + 32, :],
                                    in_=rope_sb[0][32 * h2:32 * h2 + 32, ls])
                nc.gpsimd.dma_start(out=QPl[hp2][lc][64 * i2 + 32:64 * i2 + 64, :],
                                    in_=rope_sb[1][32 * h2:32 * h2 + 32, ls])
                nc.gpsimd.dma_start(out=KPl[hp2][lc][64 * i2:64 * i2 + 32, :],
                                    in_=rope_sb[2][32 * h2:32 * h2 + 32, ls])
                nc.gpsimd.dma_start(out=KPl[hp2][lc][64 * i2 + 32:64 * i2 + 64, :],
                                    in_=rope_sb[3][32 * h2:32 * h2 + 32, ls])
        # fold 0.125*r_k into K (per head rows), in place
        for hp2 in range(2):
            nc.vector.scalar_tensor_tensor(out=KPl[hp2][lc][:],
                                           in0=KPl[hp2][lc][:], scalar=0.125,
                                           in1=Rk_b[hp2][:], op0=MUL, op1=MUL)

        # drain any remaining attention + the out-projection of fp=lc-1
        drive(ag0, 99)
        drive(ag1, 99)
        drive(fin, 99)

    # tail: fp = 3
    ag0, ag1 = att2(0, 3), att2(1, 3)
    drive(ag0, 99)
    drive(ag1, 99)
    drive(finish_fp(3), 99)


def _build_nc():
    import contextlib
    nc = bacc.Bacc("TRN2", target_bir_lowering=False, debug=False, num_devices=8)
    xT = nc.dram_tensor("xT", (D, L), BF16, kind="ExternalInput")
    wqk = nc.dram_tensor("wqk", (D, 512), BF16, kind="ExternalInput")
    wv = nc.dram_tensor("wv", (D, CPG), BF16, kind="ExternalInput")
    wo = nc.dram_tensor("wo", (CPG, D), BF16, kind="ExternalInput")
    wvec = nc.dram_tensor("wvec", (128, 32), BF16, kind="ExternalInput")
    costab = nc.dram_tensor("costab", (128, L), BF16, kind="ExternalInput")
    sintab = nc.dram_tensor("sintab", (128, L), BF16, kind="ExternalInput")
    out = nc.dram_tensor("out", (L, D), BF16, kind="ExternalOutput")
    skb36 = nc.dram_tensor("skb36", (36, L), F32)
    skbq = nc.dram_tensor("skbq", (8, L), BF16)
    skbd = nc.dram_tensor("skbd", (4, L), F32)
    dbg = (nc.dram_tensor("dbg", (8, 512), F32, kind="ExternalOutput")
           if os.environ.get("KERNEL_DBG") else None)

    with tile.TileContext(nc) as tc, contextlib.ExitStack() as ctx:
        _emit(nc, tc, ctx, xT.ap(), wqk.ap(), wv.ap(), wo.ap(), wvec.ap(),
              costab.ap(), sintab.ap(), out.ap(), skb36.ap(), skbq.ap(),
              skbd.ap(), dbg.ap() if dbg is not None else None)
    nc.compile()
    return nc


def _host_prep(x, Wqkv, Wout, q_scale, k_scale):
    x = np.asarray(x, np.float32)
    Wqkv = np.asarray(Wqkv, np.float32)
    Wout = np.asarray(Wout, np.float32)
    q_scale = np.asarray(q_scale, np.float32)
    k_scale = np.asarray(k_scale, np.float32)

    quarter = HD // 4  # 16
    inv = 1.0 / (10000.0 ** (np.arange(quarter, dtype=np.float64) / quarter))
    tt = np.repeat(np.arange(T), NP).astype(np.float64)
    pp = np.tile(np.arange(NP), T).astype(np.float64)
    ang = np.concatenate([tt[:, None] * inv[None, :], pp[:, None] * inv[None, :]],
                         axis=1)  # (L, 32)
    costab = np.tile(np.cos(ang).astype(np.float32).T, (4, 1))  # (128, L)
    sintab = np.tile(np.sin(ang).astype(np.float32).T, (4, 1))

    import ml_dtypes
    ev, od = np.arange(0, HD, 2), np.arange(1, HD, 2)
    # four [128,8] rms stationaries (qR,qI,kR,kI); q heads cols 0:4 of each
    # block, k heads cols 4:8; zero-padded so all mms share out rows 0:8
    wvec = np.zeros((128, 32), np.float32)
    for hh in range(HPG):
        r = slice(32 * hh, 32 * hh + 32)
        wvec[r, 0 + hh] = 1.0 / (HD * q_scale[ev] ** 2)
        wvec[r, 8 + hh] = 1.0 / (HD * q_scale[od] ** 2)
        wvec[r, 16 + 4 + hh] = 1.0 / (HD * k_scale[ev] ** 2)
        wvec[r, 24 + 4 + hh] = 1.0 / (HD * k_scale[od] ** 2)

    in_maps = []
    for c in range(8):
        b, g = c // 4, c % 4
        wqk = np.empty((D, 512), np.float32)
        for hh in range(HPG):
            gh = g * HPG + hh
            wq = Wqkv[gh * HD:(gh + 1) * HD, :] * q_scale[:, None]
            wk = Wqkv[D + gh * HD:D + (gh + 1) * HD, :] * k_scale[:, None]
            wqk[:, 0 + 32 * hh:32 + 32 * hh] = wq[ev].T
            wqk[:, 128 + 32 * hh:160 + 32 * hh] = wq[od].T
            wqk[:, 256 + 32 * hh:288 + 32 * hh] = wk[ev].T
            wqk[:, 384 + 32 * hh:416 + 32 * hh] = wk[od].T
        wv = np.ascontiguousarray(
            Wqkv[2 * D + g * CPG:2 * D + (g + 1) * CPG, :].T).astype(ml_dtypes.bfloat16)
        wo = np.ascontiguousarray(Wout[:, g * CPG:(g + 1) * CPG].T)
        in_maps.append({
            "xT": np.ascontiguousarray(x[b].T).astype(ml_dtypes.bfloat16),
            "wqk": wqk.astype(ml_dtypes.bfloat16), "wv": wv,
            "wo": wo.astype(ml_dtypes.bfloat16),
            "wvec": wvec.astype(ml_dtypes.bfloat16),
            "costab": costab.astype(ml_dtypes.bfloat16),
            "sintab": sintab.astype(ml_dtypes.bfloat16),
        })
    return in_maps


def kernel(x, Wqkv, Wout, q_scale, k_scale, T=None, N_p=None):
    assert int(T) == 8 and int(N_p) == 256
    if "nc" not in _CACHE:
        _CACHE["nc"] = _build_nc()
    nc = _CACHE["nc"]
    in_maps = _host_prep(x, Wqkv, Wout, q_scale, k_scale)
    trace = bool(int(os.environ.get("KERNEL_TRACE", "0")))
    res = run_bass_kernel_spmd(nc, in_maps, core_ids=list(range(8)), trace=trace)
    _CACHE["last_exec_time_ns"] = res.exec_time_ns
    outp = np.zeros((B, L, D), np.float32)
    for c in range(8):
        outp[c // 4] += np.asarray(res.results[c]["out"], np.float32)
    return outp


if __name__ == "__main__":
    rng = np.random.default_rng(0)
    x = rng.standard_normal((B, L, D), dtype=np.float32)
    Wqkv = rng.standard_normal((3 * D, D), dtype=np.float32) * 0.02
    Wout = rng.standard_normal((D, D), dtype=np.float32) * 0.02
    o = kernel(x, Wqkv, Wout, np.ones(HD, np.float32), np.ones(HD, np.float32),
               8, 256)
    print("out", o.shape, o.dtype, float(np.abs(o).mean()))


# revision 36
# speedup vs baseline: 1.1364x; 1.0727x over previous
"""Block-causal attention Trainium2 kernel (8 NeuronCores), v2.

Sharding: core c = b*4 + g handles batch b (of 2) and head-group g (4 of 16
heads). Each core computes the qkv projection, rmsnorm + 2-D RoPE,
block-causal attention and a partial output projection for its 256 channels;
the host sums the 4 per-group partials per batch (bf16 partials).

v2 changes vs baseline:
  rms sums via four zero-padded block-diagonal [128,8] stationaries -> one
    [8,512] psum bank (q heads rows 0:4, k heads rows 4:8, all mean-scaled so
    the Ln argument stays near 1 where the spline is accurate); rsqrt
    computed as exp(-0.5*ln(x+eps)) -- no sqrt table, and the 0.125 score
    scale is folded into the KPl multiply via scalar_tensor_tensor.
  normalize fused into the psum->sbuf copy (scalar_tensor_tensor with the
    broadcast reciprocal denominator, reciprocal_approx_fast on [33,512]);
    out-projection + output DMA run per-frame-pair inside the main loop
    (no serial tail); attention emitted via generators interleaved into
    phase 1 so its exp-paced stretches overlap projection matmuls.
  out, cos/sin tables, rope intermediates in bf16; shuffle DMAs on the
    sync/gpsimd queues (scalar queue kept free for the 160 exps).
Hardware pitfalls baked in: tile_position column-tiling (0,32)/(0,64)
  produces garbage on this HW (row-tiling is fine); multi-bank PSUM tiles
  misaddress the second bank; vector ops need 32-aligned partition bases and
  reciprocal_approx_fast silently fails on base-32 single-row slices.
PSUM: pps(2) + st(4x1) + pv(2) = 8 banks.
"""

import os
import numpy as np

import concourse.bass as bass
import concourse.mybir as mybir
import concourse.tile as tile
from concourse import bacc
from concourse.bass_utils import run_bass_kernel_spmd

F32 = mybir.dt.float32
BF16 = mybir.dt.bfloat16
AF = mybir.ActivationFunctionType
ALU = mybir.AluOpType
MUL = ALU.mult
ADD = ALU.add
SUB = ALU.subtract

B, T, NP, D, H = 2, 8, 256, 1024, 16
L = T * NP            # 2048
HD = 64               # head dim
HPG = 4               # heads per group (4 groups x 2 batches = 8 cores)
CPG = HPG * HD        # 256 channels per group
NDT = D // 128        # 8 d-tiles
NLC = L // 512        # 4 l-chunks (= frame pairs)
NLT = L // 128        # 16 l-tiles
EPS = 1e-6

_CACHE = {}


def _emit(nc, tc, ctx, xT, wqk, wv, wo, wvec, costab, sintab, out,
          skb36, skbq, skbd, dbg=None):
    sing = ctx.enter_context(tc.tile_pool(name="sing", bufs=1))
    xp = ctx.enter_context(tc.tile_pool(name="xp", bufs=16))
    tmp = ctx.enter_context(tc.tile_pool(name="tmp", bufs=3))
    sqp = ctx.enter_context(tc.tile_pool(name="sqp", bufs=4))
    ptp = ctx.enter_context(tc.tile_pool(name="ptp", bufs=5))
    osb = ctx.enter_context(tc.tile_pool(name="osb", bufs=2))
    bqp = ctx.enter_context(tc.tile_pool(name="bqp", bufs=4))
    rdp = ctx.enter_context(tc.tile_pool(name="rdp", bufs=2))
    # PSUM: pps 2 + st 4x1 + pv 2 = 8 banks
    pps = ctx.enter_context(tc.tile_pool(name="pps", bufs=2, space="PSUM"))
    stp = ctx.enter_context(tc.tile_pool(name="stp", bufs=4, space="PSUM"))
    pvp = ctx.enter_context(tc.tile_pool(name="pvp", bufs=2, space="PSUM"))

    # ---- persistent SBUF; first-needed first ----
    # wqk and the first l-chunk's x tiles interleaved per d-tile so the first
    # projection matmul can issue as soon as chunk 0 of each lands
    wqk_sb = sing.tile([128, NDT, 512], BF16)
    xt_pre = []
    for dt in range(NDT):
        x1 = xp.tile([128, 512], BF16, name=f"xtp{dt}", tag="xt")
        nc.sync.dma_start(out=x1[:], in_=xT[dt * 128:(dt + 1) * 128, 0:512])
        xt_pre.append(x1)
        nc.sync.dma_start(out=wqk_sb[:, dt, :],
                          in_=wqk[dt * 128:(dt + 1) * 128, :])
    wv_sb = sing.tile([128, NDT, CPG], BF16)
    nc.gpsimd.dma_start(out=wv_sb[:], in_=wv.rearrange("(t p) o -> p t o", p=128))
    wvec_sb = sing.tile([128, 32], BF16)
    nc.gpsimd.dma_start(out=wvec_sb[:], in_=wvec[:])
    cos_sb = sing.tile([128, L], BF16)
    nc.scalar.dma_start(out=cos_sb[:], in_=costab[:])
    sin_sb = sing.tile([128, L], BF16)
    nc.scalar.dma_start(out=sin_sb[:], in_=sintab[:])
    wo_sb = sing.tile([128, 2, D], BF16)
    nc.scalar.dma_start(out=wo_sb[:], in_=wo.rearrange("(t p) o -> p t o", p=128))

    qk_sb = [sing.tile([128, L], BF16, name=f"qk{i}") for i in range(4)]
    rope_sb = [sing.tile([128, L], BF16, name=f"rope{i}") for i in range(4)]
    QPl = [[sing.tile([128, 512], BF16, name=f"qp{i}_{c}") for c in range(NLC)]
           for i in range(2)]
    KPl = [[sing.tile([128, 512], BF16, name=f"kp{i}_{c}") for c in range(NLC)]
           for i in range(2)]
    v_sb = [sing.tile([128, NLT, 65], BF16, name=f"v{h}") for h in range(4)]
    att_sb = [sing.tile([128, L], BF16, name=f"att{i}") for i in range(2)]
    ones_f32 = sing.tile([128, NLT, 1], F32)
    nc.vector.memset(ones_f32[:], 1.0)
    for h in range(4):
        nc.vector.tensor_copy(v_sb[h][:, :, 64:65], ones_f32[:])


    epsb = sing.tile([8, 1], F32)
    nc.vector.memset(epsb[:], EPS)


    # ---------------- attention pass (generator, one head-pair) ------------
    def att2(hp, fp):
        nkt_sh, nkt_all = 4 * fp + 2, 4 * fp + 4
        last = nkt_all - 1
        fps = slice(512 * fp, 512 * fp + 512)
        pv = [pvp.tile([65, 512], F32, name=f"pv{hp}_{fp}_{i}", tag="pv")
              for i in range(2)]
        pend = []

        def flush_pv():
            kt_, pt_ = pend.pop(0)
            qof_ = 0 if kt_ < nkt_sh else 256
            nq_ = 512 if kt_ < nkt_sh else 256
            for i in range(2):
                nc.tensor.matmul(pv[i][:, qof_:512],
                                 v_sb[2 * hp + i][:, kt_, :],
                                 pt_[:, i, 0:nq_],
                                 start=(kt_ == 0), stop=(kt_ == last),
                                 skip_group_check=True)

        for kt in range(nkt_all):
            lck, kof = kt // 4, (kt % 4) * 128
            nq = 512 if kt < nkt_sh else 256
            qof = 0 if kt < nkt_sh else 256
            sts = [stp.tile([128, 512], F32, name=f"st{i}", tag="st")
                   for i in range(2)]
            for i in range(2):
                nc.tensor.matmul(sts[i][:, 0:nq],
                                 KPl[hp][lck][64 * i:64 * i + 64, kof:kof + 128],
                                 QPl[hp][fp][64 * i:64 * i + 64, qof:512],
                                 start=True, stop=True, skip_group_check=True)
            pt = ptp.tile([128, 2, 512], BF16, name="pt", tag="pt")
            for i in range(2):
                nc.scalar.activation(pt[:, i, 0:nq], sts[i][:, 0:nq], AF.Exp)
            pend.append((kt, pt))
            if len(pend) > 1:
                flush_pv()
            yield
        while pend:
            flush_pv()
            yield
        # extract denominators, normalize-fused copy to att_sb
        dstg = tmp.tile([33, 512], F32, tag="dc")
        nc.vector.memset(dstg[:], 1.0)
        for i in range(2):
            nc.vector.tensor_scalar(out=dstg[32 * i:32 * i + 1, :],
                                    in0=pv[i][64:65, :],
                                    scalar1=1e-30, scalar2=None, op0=ALU.max)
        dr = tmp.tile([33, 512], F32, tag="dc")
        nc.vector.reciprocal_approx_fast(out=dr[:], in_=dstg[:])
        for i in range(2):
            nc.sync.dma_start(out=skbd[2 * hp + i:2 * hp + i + 1, fps],
                              in_=dr[32 * i:32 * i + 1, :])
        for i in range(2):
            rd = rdp.tile([64, 512], F32, tag="rd")
            nc.sync.dma_start(out=rd[:],
                              in_=skbd[2 * hp + i:2 * hp + i + 1, fps]
                              .to_broadcast((64, 512)))
            nc.vector.scalar_tensor_tensor(out=att_sb[hp][64 * i:64 * i + 64, fps],
                                           in0=pv[i][0:64, :], scalar=1.0,
                                           in1=rd[:],
                                           op0=MUL, op1=MUL)
            if dbg is not None and hp == 0 and fp == 3:
                nc.sync.dma_start(out=dbg[0 + i, :], in_=dstg[32 * i:32 * i + 1, :])
                nc.sync.dma_start(out=dbg[2 + i, :], in_=dr[32 * i:32 * i + 1, :])
                nc.sync.dma_start(out=dbg[4 + i, :], in_=rd[0:1, :])
        yield

    def finish_fp(fp):
        for lt4 in range(4):
            lsl = slice((fp * 4 + lt4) * 128, (fp * 4 + lt4) * 128 + 128)
            for oc in range(2):
                ocs = slice(oc * 512, oc * 512 + 512)
                ps = pps.tile([128, 512], F32, name="ops", tag="ps")
                for ct in range(2):
                    nc.tensor.matmul(ps[:], att_sb[ct][:, lsl],
                                     wo_sb[:, ct, ocs], start=(ct == 0),
                                     stop=(ct == 1))
                ob = osb.tile([128, 512], BF16, tag="ob")
                nc.vector.tensor_copy(ob[:], ps[:])
                nc.sync.dma_start(out=out[lsl, ocs], in_=ob[:])
            yield

    def drive(gen, n):
        if gen is None:
            return False
        for _ in range(n):
            if next(gen, "END") == "END":
                return False
        return True

    # ---------------- main loop over l-chunks ------------------------------
    fin_carry = None          # finish_fp(lc-2), drained after this lc's qk pairs
    xt_next = xt_pre
    for lc in range(NLC):
        ls = slice(lc * 512, (lc + 1) * 512)
        # attention for the previous frame pair, interleaved into phase 1
        ag0 = att2(0, lc - 1) if lc >= 1 else None
        ag1 = att2(1, lc - 1) if lc >= 1 else None
        fin = finish_fp(lc - 1) if lc >= 1 else None
        nkt = 4 * lc  # kts per hp pass of fp=lc-1

        xt = xt_next

        sqs = []
        for pair in range(2):                    # 0: q (ot 0,1), 1: k (ot 2,3)
            drive(ag0, nkt // 2)
            for comp in range(2):
                ot = pair * 2 + comp
                ps = pps.tile([128, 512], F32, name="qkps", tag="ps")
                for dt in range(NDT):
                    nc.tensor.matmul(ps[:], wqk_sb[:, dt, ot * 128:(ot + 1) * 128],
                                     xt[dt][:], start=(dt == 0), stop=(dt == NDT - 1))
                nc.vector.tensor_copy(qk_sb[ot][:, ls], ps[:])
                sq = sqp.tile([128, 512], BF16, name="sq", tag="sq")
                nc.vector.tensor_tensor(sq[:], qk_sb[ot][:, ls],
                                        qk_sb[ot][:, ls], MUL)
                sqs.append(sq)
        drive(ag0, nkt // 2 + 1)
        drive(fin_carry, 99)
        # prefetch next l-chunk's x tiles
        if lc + 1 < NLC:
            nls = slice((lc + 1) * 512, (lc + 2) * 512)
            xt_next = []
            for dt in range(NDT):
                x1 = xp.tile([128, 512], BF16, name=f"xt{dt}", tag="xt")
                nc.sync.dma_start(out=x1[:], in_=xT[dt * 128:(dt + 1) * 128, nls])
                xt_next.append(x1)

        # rms sums: one [8,512] bank; q heads rows 0:4, k heads rows 4:8
        rsum = pps.tile([8, 512], F32, name="rsum", tag="ps")
        for c in range(4):
            nc.tensor.matmul(rsum[:, :], wvec_sb[:, 8 * c:8 * c + 8],
                             sqs[c][:], start=(c == 0), stop=(c == 3),
                             skip_group_check=True)
        rln = tmp.tile([8, 512], F32, tag="rln")
        nc.scalar.activation(rln[:], rsum[:], AF.Ln, bias=epsb[:])
        rqs = tmp.tile([8, 512], BF16, tag="rqs")
        nc.scalar.activation(rqs[:], rln[:], AF.Exp, scale=-0.5)
        if dbg is not None and lc == 0:
            nc.sync.dma_start(out=dbg[6, :], in_=rqs[0:1, 0:512])
            nc.sync.dma_start(out=dbg[7, :], in_=rqs[4:5, 0:512])
        nc.gpsimd.dma_start(out=skbq[0:8, ls], in_=rqs[0:8, :])
        Rq_b = bqp.tile([128, 512], BF16, tag="bq")
        for h in range(4):
            nc.gpsimd.dma_start(out=Rq_b[32 * h:32 * h + 32, :],
                                in_=skbq[h:h + 1, ls].to_broadcast((32, 512)))
        Rk_b = []
        for hp in range(2):
            rkb = bqp.tile([128, 512], BF16, name=f"rkb{hp}", tag="bq")
            for i in range(2):
                nc.gpsimd.dma_start(
                    out=rkb[64 * i:64 * i + 64, :],
                    in_=skbq[4 + 2 * hp + i:5 + 2 * hp + i, ls]
                    .to_broadcast((64, 512)))
            Rk_b.append(rkb)

        # V projection: l on partitions; 2 chains share one bank
        drive(ag1, nkt // 2)
        for vt in range(2):
            vps = pps.tile([128, 2, CPG], F32, name="vps", tag="ps")
            for c2 in range(2):
                ls4 = vt * 2 + c2
                for dt in range(NDT):
                    nc.tensor.matmul(vps[:, c2, :],
                                     xt[dt][:, ls4 * 128:(ls4 + 1) * 128],
                                     wv_sb[:, dt, :],
                                     start=(c2 == 0 and dt == 0),
                                     stop=(c2 == 1 and dt == NDT - 1),
                                     skip_group_check=True)
            for c2 in range(2):
                lt = lc * 4 + vt * 2 + c2
                for h in range(4):
                    nc.vector.tensor_copy(v_sb[h][:, lt, 0:64],
                                          vps[:, c2, h * 64:(h + 1) * 64])
        drive(ag1, nkt // 2 + 1)
        # per-lc RoPE (+ r_q fold on the q side), all bf16
        for base in (0, 2):
            xr, xi = qk_sb[base][:, ls], qk_sb[base + 1][:, ls]
            for comp in range(2):
                t1 = tmp.tile([128, 512], BF16, tag="t1")
                t2 = tmp.tile([128, 512], BF16, tag="t2")
                ca, cb = (cos_sb, sin_sb) if comp == 0 else (sin_sb, cos_sb)
                nc.vector.tensor_tensor(t1[:], xr, ca[:, ls], MUL)
                nc.vector.tensor_tensor(t2[:], xi, cb[:, ls], MUL)
                op = SUB if comp == 0 else ADD
                dst = rope_sb[base + comp][:, ls]
                if base == 0:
                    t3 = tmp.tile([128, 512], BF16, tag="t3")
                    nc.vector.tensor_tensor(t3[:], t1[:], t2[:], op)
                    nc.vector.tensor_tensor(dst, t3[:], Rq_b[:], MUL)
                else:
                    nc.vector.tensor_tensor(dst, t1[:], t2[:], op)

        # shuffle into per-head contiguous tiles (sync queue)
        for hp2 in range(2):
            for i2 in range(2):
                h2 = hp2 * 2 + i2
                nc.gpsimd.dma_start(out=QPl[hp2][lc][64 * i2:64 * i2 + 32, :],
                                    in_=rope_sb[0][32 * h2:32 * h2 + 32, ls])
                nc.gpsimd.dma_start(out=QPl[hp2][lc][64 * i2 + 32:64 * i2 + 64, :],
                                    in_=rope_sb[1][32 * h2:32 * h2 + 32, ls])
                nc.gpsimd.dma_start(out=KPl[hp2][lc][64 * i2:64 * i2 + 32, :],
                                    in_=rope_sb[2][32 * h2:32 * h2 + 32, ls])
                nc.gpsimd.dma_start(out=KPl[hp2][lc][64 * i2 + 32:64 * i2 + 64, :],
                                    in_=rope_sb[3][32 * h2:32 * h2 + 32, ls])
        # fold 0.125*r_k into K (per head rows), in place
        for hp2 in range(2):
            nc.vector.scalar_tensor_tensor(out=KPl[hp2][lc][:],
                                           in0=KPl[hp2][lc][:], scalar=0.125,
                                           in1=Rk_b[hp2][:], op0=MUL, op1=MUL)

        # drain any remaining attention; fin is carried into the next lc
        drive(ag0, 99)
        drive(ag1, 99)
        fin_carry = fin

    # tail: fp = 3 (fin(2) fills the tail's score/exp warmup)
    ag0, ag1 = att2(0, 3), att2(1, 3)
    drive(ag0, 4)
    drive(fin_carry, 99)
    drive(ag0, 99)
    drive(ag1, 99)
    drive(finish_fp(3), 99)


def _build_nc():
    import contextlib
    nc = bacc.Bacc("TRN2", target_bir_lowering=False, debug=False, num_devices=8)
    xT = nc.dram_tensor("xT", (D, L), BF16, kind="ExternalInput")
    wqk = nc.dram_tensor("wqk", (D, 512), BF16, kind="ExternalInput")
    wv = nc.dram_tensor("wv", (D, CPG), BF16, kind="ExternalInput")
    wo = nc.dram_tensor("wo", (CPG, D), BF16, kind="ExternalInput")
    wvec = nc.dram_tensor("wvec", (128, 32), BF16, kind="ExternalInput")
    costab = nc.dram_tensor("costab", (128, L), BF16, kind="ExternalInput")
    sintab = nc.dram_tensor("sintab", (128, L), BF16, kind="ExternalInput")
    out = nc.dram_tensor("out", (L, D), BF16, kind="ExternalOutput")
    skb36 = nc.dram_tensor("skb36", (36, L), F32)
    skbq = nc.dram_tensor("skbq", (8, L), BF16)
    skbd = nc.dram_tensor("skbd", (4, L), F32)
    dbg = (nc.dram_tensor("dbg", (8, 512), F32, kind="ExternalOutput")
           if os.environ.get("KERNEL_DBG") else None)

    with tile.TileContext(nc) as tc, contextlib.ExitStack() as ctx:
        _emit(nc, tc, ctx, xT.ap(), wqk.ap(), wv.ap(), wo.ap(), wvec.ap(),
              costab.ap(), sintab.ap(), out.ap(), skb36.ap(), skbq.ap(),
              skbd.ap(), dbg.ap() if dbg is not None else None)
    nc.compile()
    return nc


def _host_prep(x, Wqkv, Wout, q_scale, k_scale):
    x = np.asarray(x, np.float32)
    Wqkv = np.asarray(Wqkv, np.float32)
    Wout = np.asarray(Wout, np.float32)
    q_scale = np.asarray(q_scale, np.float32)
    k_scale = np.asarray(k_scale, np.float32)

    quarter = HD // 4  # 16
    inv = 1.0 / (10000.0 ** (np.arange(quarter, dtype=np.float64) / quarter))
    tt = np.repeat(np.arange(T), NP).astype(np.float64)
    pp = np.tile(np.arange(NP), T).astype(np.float64)
    ang = np.concatenate([tt[:, None] * inv[None, :], pp[:, None] * inv[None, :]],
                         axis=1)  # (L, 32)
    costab = np.tile(np.cos(ang).astype(np.float32).T, (4, 1))  # (128, L)
    sintab = np.tile(np.sin(ang).astype(np.float32).T, (4, 1))

    import ml_dtypes
    ev, od = np.arange(0, HD, 2), np.arange(1, HD, 2)
    # four [128,8] rms stationaries (qR,qI,kR,kI); q heads cols 0:4 of each
    # block, k heads cols 4:8; zero-padded so all mms share out rows 0:8
    wvec = np.zeros((128, 32), np.float32)
    for hh in range(HPG):
        r = slice(32 * hh, 32 * hh + 32)
        wvec[r, 0 + hh] = 1.0 / (HD * q_scale[ev] ** 2)
        wvec[r, 8 + hh] = 1.0 / (HD * q_scale[od] ** 2)
        wvec[r, 16 + 4 + hh] = 1.0 / (HD * k_scale[ev] ** 2)
        wvec[r, 24 + 4 + hh] = 1.0 / (HD * k_scale[od] ** 2)

    in_maps = []
    for c in range(8):
        b, g = c // 4, c % 4
        wqk = np.empty((D, 512), np.float32)
        for hh in range(HPG):
            gh = g * HPG + hh
            wq = Wqkv[gh * HD:(gh + 1) * HD, :] * q_scale[:, None]
            wk = Wqkv[D + gh * HD:D + (gh + 1) * HD, :] * k_scale[:, None]
            wqk[:, 0 + 32 * hh:32 + 32 * hh] = wq[ev].T
            wqk[:, 128 + 32 * hh:160 + 32 * hh] = wq[od].T
            wqk[:, 256 + 32 * hh:288 + 32 * hh] = wk[ev].T
            wqk[:, 384 + 32 * hh:416 + 32 * hh] = wk[od].T
        wv = np.ascontiguousarray(
            Wqkv[2 * D + g * CPG:2 * D + (g + 1) * CPG, :].T).astype(ml_dtypes.bfloat16)
        wo = np.ascontiguousarray(Wout[:, g * CPG:(g + 1) * CPG].T)
        in_maps.append({
            "xT": np.ascontiguousarray(x[b].T).astype(ml_dtypes.bfloat16),
            "wqk": wqk.astype(ml_dtypes.bfloat16), "wv": wv,
            "wo": wo.astype(ml_dtypes.bfloat16),
            "wvec": wvec.astype(ml_dtypes.bfloat16),
            "costab": costab.astype(ml_dtypes.bfloat16),
            "sintab": sintab.astype(ml_dtypes.bfloat16),
        })
    return in_maps


def kernel(x, Wqkv, Wout, q_scale, k_scale, T=None, N_p=None):
    assert int(T) == 8 and int(N_p) == 256
    if "nc" not in _CACHE:
        _CACHE["nc"] = _build_nc()
    nc = _CACHE["nc"]
    in_maps = _host_prep(x, Wqkv, Wout, q_scale, k_scale)
    trace = bool(int(os.environ.get("KERNEL_TRACE", "0")))
    res = run_bass_kernel_spmd(nc, in_maps, core_ids=list(range(8)), trace=trace)
    _CACHE["last_exec_time_ns"] = res.exec_time_ns
    outp = np.zeros((B, L, D), np.float32)
    for c in range(8):
        outp[c // 4] += np.asarray(res.results[c]["out"], np.float32)
    return outp


if __name__ == "__main__":
    rng = np.random.default_rng(0)
    x = rng.standard_normal((B, L, D), dtype=np.float32)
    Wqkv = rng.standard_normal((3 * D, D), dtype=np.float32) * 0.02
    Wout = rng.standard_normal((D, D), dtype=np.float32) * 0.02
    o = kernel(x, Wqkv, Wout, np.ones(HD, np.float32), np.ones(HD, np.float32),
               8, 256)
    print("out", o.shape, o.dtype, float(np.abs(o).mean()))
